# revision 8
# baseline (speedup 1.0000x reference)
"""Trainium2 Bass kernel for nn_FFTChainMatrix (block-circulant matmul via 64-pt rFFT).

v2: dense 2-blocks-per-128-partitions factorization.

y = x @ W.T with W 4096x4096 block-circulant (64x64 grid of 64x64 circulant
blocks).  FFT-domain pipeline per 512-token core shard, all tiles [128, *]:

  load    x feature-major, block-pair-major: xt[(b,c), j*T+t]    (1 streaming DMA x4)
  S1      rfft per block pair:  X1_j = A2^T @ xt_j     (32 MM, shared stationary)
  shuf    (b,phi)-major -> freq-major:  32 strided SBUF DMAs
  S2      per-freq complex contraction over blocks: Y2_f = G_f^T @ X2_f   (32 MM)
  unshuf  freq-major -> block-pair-major: 32 strided SBUF DMAs
  S3      irfft per block pair: y_j = B2^T @ Y3_j      (32 MM, shared stationary)
  store   y feature-major [128, 32*T] -> HBM; host un-permutes/transposes.

Sharding: data-parallel over tokens, 4096 tokens -> 8 cores x 512.
"""

from contextlib import ExitStack

import numpy as np

BLK = 64
NB = 64           # blocks per side
T = 512           # tokens per core
NCORES = 8
FEAT = 4096
NJ = 32           # block pairs


# ---------------------------------------------------------------- host math
def _build_afft():
    r = np.arange(BLK)
    A = np.zeros((BLK, BLK))
    A[0, :] = 1.0
    A[1, :] = (-1.0) ** r
    for p in range(1, 32):
        A[2 * p, :] = np.cos(2 * np.pi * p * r / BLK)
        A[2 * p + 1, :] = -np.sin(2 * np.pi * p * r / BLK)
    return A


def _build_bifft():
    r = np.arange(BLK)
    B = np.zeros((BLK, BLK))
    B[:, 0] = 1.0 / BLK
    B[:, 1] = ((-1.0) ** r) / BLK
    for p in range(1, 32):
        B[:, 2 * p] = 2.0 * np.cos(2 * np.pi * p * r / BLK) / BLK
        B[:, 2 * p + 1] = -2.0 * np.sin(2 * np.pi * p * r / BLK) / BLK
    return B


def _build_matrices(circulant_params, channel_weights):
    """A2 [128,128], G [32,128,128], B2 [128,128] float64."""
    c_w = np.einsum("m,moid->oid",
                    np.asarray(channel_weights, np.float64),
                    np.asarray(circulant_params, np.float64))
    Chat = np.fft.rfft(c_w, axis=-1)
    Wr, Wi = Chat.real, Chat.imag

    Afft = _build_afft()
    Bifft = _build_bifft()
    # S1 output partitions phi-major: A2[b*64+c, 2*phi+b] = Afft[phi, c]
    A2 = np.zeros((128, 128))
    for b in range(2):
        A2[b * 64:(b + 1) * 64, b::2] = Afft.T
    # S3 input partitions phi-major: B2[2*phi+b, b*64+d] = Bifft[d, phi]
    B2 = np.zeros((128, 128))
    for b in range(2):
        B2[b::2, b * 64:(b + 1) * 64] = Bifft.T

    # (component c1, block i=2j+b) -> x2/y2 partition c1*64 + b*32 + j
    i_ = np.arange(NB)
    pre = (i_ % 2) * 32 + i_ // 2
    pim = pre + 64
    G = np.zeros((32, 128, 128))
    for f in range(32):
        if f == 0:
            for i in range(NB):
                G[0, pre[i], pre] = Wr[:, i, 0]
                G[0, pim[i], pim] = Wr[:, i, 32]
        else:
            for i in range(NB):
                G[f, pre[i], pre] = Wr[:, i, f]
                G[f, pim[i], pre] = -Wi[:, i, f]
                G[f, pre[i], pim] = Wi[:, i, f]
                G[f, pim[i], pim] = Wr[:, i, f]
    return A2, G, B2


# ---------------------------------------------------------------- bass trace
def _trace_nc():
    import concourse.bass as bass  # noqa: F401
    import concourse.mybir as mybir
    import concourse.tile as tile
    from concourse import bacc

    f32 = mybir.dt.float32
    f16 = mybir.dt.float16

    nc = bacc.Bacc("TRN2", target_bir_lowering=False, debug=False,
                   num_devices=NCORES)
    x_h = nc.dram_tensor("x_shard", [128, NJ * T], f16, kind="ExternalInput").ap()
    a_h = nc.dram_tensor("a2_mat", [128, 128], f16, kind="ExternalInput").ap()
    g_h = nc.dram_tensor("g_mats", [128, 32 * 128], f16, kind="ExternalInput").ap()
    b_h = nc.dram_tensor("b2_mat", [128, 128], f16, kind="ExternalInput").ap()
    y_h = nc.dram_tensor("y_shard", [128, NJ * T], f16, kind="ExternalOutput").ap()

    cp_ix = [0]
    shuf_ix = [0]
    unshuf_ix = [0]

    with tile.TileContext(nc) as tc, ExitStack() as ctx:
        wpool = ctx.enter_context(tc.tile_pool(name="weights", bufs=1))
        dpool = ctx.enter_context(tc.tile_pool(name="data", bufs=1))
        y2p = ctx.enter_context(tc.tile_pool(name="y2p", bufs=4))
        mm_ps = ctx.enter_context(tc.tile_pool(name="mm_ps", bufs=2, space="PSUM"))
        s2_ps = ctx.enter_context(tc.tile_pool(name="s2_ps", bufs=2, space="PSUM"))

        def copyback(out_ap, in_ap):
            # PSUM readers: DVE + Act only
            if cp_ix[0] % 2 == 0:
                nc.vector.tensor_copy(out_ap, in_ap)
            else:
                nc.scalar.copy(out_ap, in_ap)
            cp_ix[0] += 1

        def shuf_dma(dst, src):
            eng = (nc.sync, nc.gpsimd)[shuf_ix[0] % 2]
            shuf_ix[0] += 1
            return eng.dma_start(dst, src)

        def unshuf_dma(dst, src):
            eng = (nc.gpsimd, nc.sync)[unshuf_ix[0] % 2]
            unshuf_ix[0] += 1
            return eng.dma_start(dst, src)

        a2t = wpool.tile([128, 128], f16)
        nc.gpsimd.dma_start(a2t[:], a_h[:])
        gt = wpool.tile([128, 32 * 128], f16)
        nc.gpsimd.dma_start(gt[:], g_h[:])
        b2t = wpool.tile([128, 128], f16)
        nc.gpsimd.dma_start(b2t[:], b_h[:])

        # ---- load x (4 streaming chunks, 8 block-pairs each)
        xt = dpool.tile([128, NJ * T], f16, tag="xt")
        for ch in range(4):
            nc.sync.dma_start(xt[:, ch * 8 * T:(ch + 1) * 8 * T],
                              x_h[:, ch * 8 * T:(ch + 1) * 8 * T])

        # ---- S1: 32 matmuls, shared stationary A2; 2-bank merged copybacks
        x1 = dpool.tile([128, NJ * T], f16, tag="x1")
        for jp in range(NJ // 2):
            ps = mm_ps.tile([128, 2 * T], f32, tag="mm")
            nc.tensor.matmul(ps[:, :T], a2t[:], xt[:, (2 * jp) * T:(2 * jp + 1) * T],
                             start=True, stop=True)
            nc.tensor.matmul(ps[:, T:], a2t[:], xt[:, (2 * jp + 1) * T:(2 * jp + 2) * T],
                             start=True, stop=True)
            copyback(x1[:, (2 * jp) * T:(2 * jp + 2) * T], ps[:])

        # ---- shuffle: x1 partitions are phi-major (p = 2*phi + b), so each
        # freq-pair f reads 4 contiguous partitions [4f, 4f+4) fanned out to
        # 128 dst partitions (c1*64 + b*32 + j).
        x2 = dpool.tile([128, NJ * T], f16, tag="x2")
        for f in range(32):
            shuf_dma(x2[:, f * T:(f + 1) * T], x1[4 * f:4 * f + 4, :])

        # ---- S2 + unshuffle (y3 partitions phi-major: p = 2*phi + b2)
        y3 = dpool.tile([128, NJ * T], f16, tag="y3")
        for fp in range(16):
            f0, f1 = 2 * fp, 2 * fp + 1
            ps = s2_ps.tile([128, 2 * T], f32, tag="s2")
            nc.tensor.matmul(ps[:, :T], gt[:, f0 * 128:(f0 + 1) * 128],
                             x2[:, f0 * T:(f0 + 1) * T], start=True, stop=True)
            nc.tensor.matmul(ps[:, T:], gt[:, f1 * 128:(f1 + 1) * 128],
                             x2[:, f1 * T:(f1 + 1) * T], start=True, stop=True)
            y2c = y2p.tile([128, 2 * T], f16, tag="y2")
            copyback(y2c[:], ps[:])
            unshuf_dma(y3[4 * f0:4 * f0 + 4, :], y2c[:, :T])
            unshuf_dma(y3[4 * f1:4 * f1 + 4, :], y2c[:, T:])

        # ---- S3: 32 matmuls, shared stationary B2; store
        yo = dpool.tile([128, NJ * T], f16, tag="yo")
        for jp in range(NJ // 2):
            ps = mm_ps.tile([128, 2 * T], f32, tag="mm")
            nc.tensor.matmul(ps[:, :T], b2t[:], y3[:, (2 * jp) * T:(2 * jp + 1) * T],
                             start=True, stop=True)
            nc.tensor.matmul(ps[:, T:], b2t[:], y3[:, (2 * jp + 1) * T:(2 * jp + 2) * T],
                             start=True, stop=True)
            copyback(yo[:, (2 * jp) * T:(2 * jp + 2) * T], ps[:])
        for ch in range(4):
            nc.sync.dma_start(y_h[:, ch * 8 * T:(ch + 1) * 8 * T],
                              yo[:, ch * 8 * T:(ch + 1) * 8 * T])

    nc.compile()
    return nc


_CACHE = {}


def make_in_maps(x, circulant_params, channel_weights):
    xf = np.ascontiguousarray(np.asarray(x, np.float32)).reshape(-1, FEAT)
    assert xf.shape[0] == NCORES * T, f"unexpected token count {xf.shape}"
    A2, G, B2 = _build_matrices(circulant_params, channel_weights)
    a2_f16 = A2.astype(np.float16)
    b2_f16 = B2.astype(np.float16)
    g_f16 = np.ascontiguousarray(
        G.transpose(1, 0, 2).reshape(128, 32 * 128).astype(np.float16))
    xf16 = xf.astype(np.float16)
    maps = []
    for c in range(NCORES):
        xs = xf16[c * T:(c + 1) * T].T                       # [4096, 512]
        xt = np.ascontiguousarray(
            xs.reshape(NJ, 2, BLK, T).transpose(1, 2, 0, 3).reshape(128, NJ * T))
        maps.append({
            "x_shard": xt,
            "a2_mat": a2_f16,
            "g_mats": g_f16,
            "b2_mat": b2_f16,
        })
    return maps


def kernel(x, circulant_params, channel_weights):
    from concourse.bass_utils import run_bass_kernel_spmd

    x = np.ascontiguousarray(np.asarray(x, np.float32))
    orig_shape = x.shape

    if "nc" not in _CACHE:
        _CACHE["nc"] = _trace_nc()
    nc = _CACHE["nc"]

    in_maps = make_in_maps(x, circulant_params, channel_weights)
    res = run_bass_kernel_spmd(nc, in_maps, core_ids=list(range(NCORES)))
    outs = []
    for c in range(NCORES):
        yo = res.results[c]["y_shard"]                       # [128, 32*T] f16
        yv = yo.reshape(2, BLK, NJ, T).transpose(3, 2, 0, 1).reshape(T, FEAT)
        outs.append(yv)
    y = np.concatenate(outs, axis=0)
    return y.astype(np.float32).reshape(orig_shape)


# revision 10
# speedup vs baseline: 1.0038x; 1.0038x over previous
"""Trainium2 Bass kernel for nn_FFTChainMatrix (block-circulant matmul via 64-pt rFFT).

v2: dense 2-blocks-per-128-partitions factorization.

y = x @ W.T with W 4096x4096 block-circulant (64x64 grid of 64x64 circulant
blocks).  FFT-domain pipeline per 512-token core shard, all tiles [128, *]:

  load    x feature-major, block-pair-major: xt[(b,c), j*T+t]    (1 streaming DMA x4)
  S1      rfft per block pair:  X1_j = A2^T @ xt_j     (32 MM, shared stationary)
  shuf    (b,phi)-major -> freq-major:  32 strided SBUF DMAs
  S2      per-freq complex contraction over blocks: Y2_f = G_f^T @ X2_f   (32 MM)
  unshuf  freq-major -> block-pair-major: 32 strided SBUF DMAs
  S3      irfft per block pair: y_j = B2^T @ Y3_j      (32 MM, shared stationary)
  store   y feature-major [128, 32*T] -> HBM; host un-permutes/transposes.

Sharding: data-parallel over tokens, 4096 tokens -> 8 cores x 512.
"""

from contextlib import ExitStack

import numpy as np

BLK = 64
NB = 64           # blocks per side
T = 512           # tokens per core
NCORES = 8
FEAT = 4096
NJ = 32           # block pairs


# ---------------------------------------------------------------- host math
def _build_afft():
    r = np.arange(BLK)
    A = np.zeros((BLK, BLK))
    A[0, :] = 1.0
    A[1, :] = (-1.0) ** r
    for p in range(1, 32):
        A[2 * p, :] = np.cos(2 * np.pi * p * r / BLK)
        A[2 * p + 1, :] = -np.sin(2 * np.pi * p * r / BLK)
    return A


def _build_bifft():
    r = np.arange(BLK)
    B = np.zeros((BLK, BLK))
    B[:, 0] = 1.0 / BLK
    B[:, 1] = ((-1.0) ** r) / BLK
    for p in range(1, 32):
        B[:, 2 * p] = 2.0 * np.cos(2 * np.pi * p * r / BLK) / BLK
        B[:, 2 * p + 1] = -2.0 * np.sin(2 * np.pi * p * r / BLK) / BLK
    return B


def _build_matrices(circulant_params, channel_weights):
    """A2 [128,128], G [32,128,128], B2 [128,128] float64."""
    c_w = np.einsum("m,moid->oid",
                    np.asarray(channel_weights, np.float64),
                    np.asarray(circulant_params, np.float64))
    Chat = np.fft.rfft(c_w, axis=-1)
    Wr, Wi = Chat.real, Chat.imag

    Afft = _build_afft()
    Bifft = _build_bifft()
    # S1 output partitions phi-major: A2[b*64+c, 2*phi+b] = Afft[phi, c]
    A2 = np.zeros((128, 128))
    for b in range(2):
        A2[b * 64:(b + 1) * 64, b::2] = Afft.T
    # S3 input partitions phi-major: B2[2*phi+b, b*64+d] = Bifft[d, phi]
    B2 = np.zeros((128, 128))
    for b in range(2):
        B2[b::2, b * 64:(b + 1) * 64] = Bifft.T

    # (component c1, block i=2j+b) -> x2/y2 partition c1*64 + b*32 + j
    i_ = np.arange(NB)
    pre = (i_ % 2) * 32 + i_ // 2
    pim = pre + 64
    G = np.zeros((32, 128, 128))
    for f in range(32):
        if f == 0:
            for i in range(NB):
                G[0, pre[i], pre] = Wr[:, i, 0]
                G[0, pim[i], pim] = Wr[:, i, 32]
        else:
            for i in range(NB):
                G[f, pre[i], pre] = Wr[:, i, f]
                G[f, pim[i], pre] = -Wi[:, i, f]
                G[f, pre[i], pim] = Wi[:, i, f]
                G[f, pim[i], pim] = Wr[:, i, f]
    return A2, G, B2


# ---------------------------------------------------------------- bass trace
def _trace_nc():
    import concourse.bass as bass  # noqa: F401
    import concourse.mybir as mybir
    import concourse.tile as tile
    from concourse import bacc

    f32 = mybir.dt.float32
    f16 = mybir.dt.float16

    nc = bacc.Bacc("TRN2", target_bir_lowering=False, debug=False,
                   num_devices=NCORES)
    x_h = nc.dram_tensor("x_shard", [128, NJ * T], f16, kind="ExternalInput").ap()
    a_h = nc.dram_tensor("a2_mat", [128, 128], f16, kind="ExternalInput").ap()
    g_h = nc.dram_tensor("g_mats", [128, 32 * 128], f16, kind="ExternalInput").ap()
    b_h = nc.dram_tensor("b2_mat", [128, 128], f16, kind="ExternalInput").ap()
    y_h = nc.dram_tensor("y_shard", [128, NJ * T], f16, kind="ExternalOutput").ap()

    cp_ix = [0]
    shuf_ix = [0]
    unshuf_ix = [0]

    with tile.TileContext(nc) as tc, ExitStack() as ctx:
        wpool = ctx.enter_context(tc.tile_pool(name="weights", bufs=1))
        dpool = ctx.enter_context(tc.tile_pool(name="data", bufs=1))
        y2p = ctx.enter_context(tc.tile_pool(name="y2p", bufs=4))
        mm_ps = ctx.enter_context(tc.tile_pool(name="mm_ps", bufs=2, space="PSUM"))
        s2_ps = ctx.enter_context(tc.tile_pool(name="s2_ps", bufs=2, space="PSUM"))

        def copyback(out_ap, in_ap):
            # PSUM readers: DVE + Act only
            if cp_ix[0] % 2 == 0:
                nc.vector.tensor_copy(out_ap, in_ap)
            else:
                nc.scalar.copy(out_ap, in_ap)
            cp_ix[0] += 1

        def shuf_dma(dst, src):
            eng = (nc.sync, nc.scalar, nc.gpsimd)[shuf_ix[0] % 3]
            shuf_ix[0] += 1
            return eng.dma_start(dst, src)

        def unshuf_dma(dst, src):
            eng = (nc.gpsimd, nc.sync, nc.scalar)[unshuf_ix[0] % 3]
            unshuf_ix[0] += 1
            return eng.dma_start(dst, src)

        a2t = wpool.tile([128, 128], f16)
        nc.gpsimd.dma_start(a2t[:], a_h[:])
        gt = wpool.tile([128, 32 * 128], f16)
        nc.scalar.dma_start(gt[:], g_h[:])
        b2t = wpool.tile([128, 128], f16)
        nc.gpsimd.dma_start(b2t[:], b_h[:])

        # ---- load x (8 streaming chunks, 4 block-pairs each, 3 queues)
        xt = dpool.tile([128, NJ * T], f16, tag="xt")
        for ch in range(8):
            eng = (nc.sync, nc.scalar, nc.gpsimd)[ch % 3]
            eng.dma_start(xt[:, ch * 4 * T:(ch + 1) * 4 * T],
                          x_h[:, ch * 4 * T:(ch + 1) * 4 * T])

        # ---- S1: 32 matmuls, shared stationary A2; 2-bank merged copybacks
        x1 = dpool.tile([128, NJ * T], f16, tag="x1")
        for jp in range(NJ // 2):
            ps = mm_ps.tile([128, 2 * T], f32, tag="mm")
            nc.tensor.matmul(ps[:, :T], a2t[:], xt[:, (2 * jp) * T:(2 * jp + 1) * T],
                             start=True, stop=True)
            nc.tensor.matmul(ps[:, T:], a2t[:], xt[:, (2 * jp + 1) * T:(2 * jp + 2) * T],
                             start=True, stop=True)
            copyback(x1[:, (2 * jp) * T:(2 * jp + 2) * T], ps[:])

        # ---- shuffle: x1 partitions are phi-major (p = 2*phi + b), so each
        # freq-pair f reads 4 contiguous partitions [4f, 4f+4) fanned out to
        # 128 dst partitions (c1*64 + b*32 + j).
        x2 = dpool.tile([128, NJ * T], f16, tag="x2")
        for f in range(32):
            shuf_dma(x2[:, f * T:(f + 1) * T], x1[4 * f:4 * f + 4, :])

        # ---- S2 + unshuffle (y3 partitions phi-major: p = 2*phi + b2)
        y3 = dpool.tile([128, NJ * T], f16, tag="y3")
        for fp in range(16):
            f0, f1 = 2 * fp, 2 * fp + 1
            ps = s2_ps.tile([128, 2 * T], f32, tag="s2")
            nc.tensor.matmul(ps[:, :T], gt[:, f0 * 128:(f0 + 1) * 128],
                             x2[:, f0 * T:(f0 + 1) * T], start=True, stop=True)
            nc.tensor.matmul(ps[:, T:], gt[:, f1 * 128:(f1 + 1) * 128],
                             x2[:, f1 * T:(f1 + 1) * T], start=True, stop=True)
            y2c = y2p.tile([128, 2 * T], f16, tag="y2")
            copyback(y2c[:], ps[:])
            unshuf_dma(y3[4 * f0:4 * f0 + 4, :], y2c[:, :T])
            unshuf_dma(y3[4 * f1:4 * f1 + 4, :], y2c[:, T:])

        # ---- S3: 32 matmuls, shared stationary B2; store
        yo = dpool.tile([128, NJ * T], f16, tag="yo")
        for jp in range(NJ // 2):
            ps = mm_ps.tile([128, 2 * T], f32, tag="mm")
            nc.tensor.matmul(ps[:, :T], b2t[:], y3[:, (2 * jp) * T:(2 * jp + 1) * T],
                             start=True, stop=True)
            nc.tensor.matmul(ps[:, T:], b2t[:], y3[:, (2 * jp + 1) * T:(2 * jp + 2) * T],
                             start=True, stop=True)
            copyback(yo[:, (2 * jp) * T:(2 * jp + 2) * T], ps[:])
        for ch in range(8):
            eng = (nc.sync, nc.scalar, nc.gpsimd)[ch % 3]
            eng.dma_start(y_h[:, ch * 4 * T:(ch + 1) * 4 * T],
                          yo[:, ch * 4 * T:(ch + 1) * 4 * T])

    nc.compile()
    return nc


_CACHE = {}


def make_in_maps(x, circulant_params, channel_weights):
    xf = np.ascontiguousarray(np.asarray(x, np.float32)).reshape(-1, FEAT)
    assert xf.shape[0] == NCORES * T, f"unexpected token count {xf.shape}"
    A2, G, B2 = _build_matrices(circulant_params, channel_weights)
    a2_f16 = A2.astype(np.float16)
    b2_f16 = B2.astype(np.float16)
    g_f16 = np.ascontiguousarray(
        G.transpose(1, 0, 2).reshape(128, 32 * 128).astype(np.float16))
    xf16 = xf.astype(np.float16)
    maps = []
    for c in range(NCORES):
        xs = xf16[c * T:(c + 1) * T].T                       # [4096, 512]
        xt = np.ascontiguousarray(
            xs.reshape(NJ, 2, BLK, T).transpose(1, 2, 0, 3).reshape(128, NJ * T))
        maps.append({
            "x_shard": xt,
            "a2_mat": a2_f16,
            "g_mats": g_f16,
            "b2_mat": b2_f16,
        })
    return maps


def kernel(x, circulant_params, channel_weights):
    from concourse.bass_utils import run_bass_kernel_spmd

    x = np.ascontiguousarray(np.asarray(x, np.float32))
    orig_shape = x.shape

    if "nc" not in _CACHE:
        _CACHE["nc"] = _trace_nc()
    nc = _CACHE["nc"]

    in_maps = make_in_maps(x, circulant_params, channel_weights)
    res = run_bass_kernel_spmd(nc, in_maps, core_ids=list(range(NCORES)))
    outs = []
    for c in range(NCORES):
        yo = res.results[c]["y_shard"]                       # [128, 32*T] f16
        yv = yo.reshape(2, BLK, NJ, T).transpose(3, 2, 0, 1).reshape(T, FEAT)
        outs.append(yv)
    y = np.concatenate(outs, axis=0)
    return y.astype(np.float32).reshape(orig_shape)


# revision 11
# speedup vs baseline: 1.0168x; 1.0129x over previous
"""Trainium2 Bass kernel for nn_FFTChainMatrix (block-circulant matmul via 64-pt rFFT).

v2: dense 2-blocks-per-128-partitions factorization.

y = x @ W.T with W 4096x4096 block-circulant (64x64 grid of 64x64 circulant
blocks).  FFT-domain pipeline per 512-token core shard, all tiles [128, *]:

  load    x feature-major, block-pair-major: xt[(b,c), j*T+t]    (1 streaming DMA x4)
  S1      rfft per block pair:  X1_j = A2^T @ xt_j     (32 MM, shared stationary)
  shuf    (b,phi)-major -> freq-major:  32 strided SBUF DMAs
  S2      per-freq complex contraction over blocks: Y2_f = G_f^T @ X2_f   (32 MM)
  unshuf  freq-major -> block-pair-major: 32 strided SBUF DMAs
  S3      irfft per block pair: y_j = B2^T @ Y3_j      (32 MM, shared stationary)
  store   y feature-major [128, 32*T] -> HBM; host un-permutes/transposes.

Sharding: data-parallel over tokens, 4096 tokens -> 8 cores x 512.
"""

from contextlib import ExitStack

import numpy as np

BLK = 64
NB = 64           # blocks per side
T = 512           # tokens per core
NCORES = 8
FEAT = 4096
NJ = 32           # block pairs


# ---------------------------------------------------------------- host math
def _build_afft():
    r = np.arange(BLK)
    A = np.zeros((BLK, BLK))
    A[0, :] = 1.0
    A[1, :] = (-1.0) ** r
    for p in range(1, 32):
        A[2 * p, :] = np.cos(2 * np.pi * p * r / BLK)
        A[2 * p + 1, :] = -np.sin(2 * np.pi * p * r / BLK)
    return A


def _build_bifft():
    r = np.arange(BLK)
    B = np.zeros((BLK, BLK))
    B[:, 0] = 1.0 / BLK
    B[:, 1] = ((-1.0) ** r) / BLK
    for p in range(1, 32):
        B[:, 2 * p] = 2.0 * np.cos(2 * np.pi * p * r / BLK) / BLK
        B[:, 2 * p + 1] = -2.0 * np.sin(2 * np.pi * p * r / BLK) / BLK
    return B


def _build_matrices(circulant_params, channel_weights):
    """A2 [128,128], G [32,128,128], B2 [128,128] float64."""
    c_w = np.einsum("m,moid->oid",
                    np.asarray(channel_weights, np.float64),
                    np.asarray(circulant_params, np.float64))
    Chat = np.fft.rfft(c_w, axis=-1)
    Wr, Wi = Chat.real, Chat.imag

    Afft = _build_afft()
    Bifft = _build_bifft()
    # S1 output partitions phi-major: A2[b*64+c, 2*phi+b] = Afft[phi, c]
    A2 = np.zeros((128, 128))
    for b in range(2):
        A2[b * 64:(b + 1) * 64, b::2] = Afft.T
    # S3 input partitions phi-major: B2[2*phi+b, b*64+d] = Bifft[d, phi]
    B2 = np.zeros((128, 128))
    for b in range(2):
        B2[b::2, b * 64:(b + 1) * 64] = Bifft.T

    # (component c1, block i=2j+b) -> x2/y2 partition c1*64 + b*32 + j
    i_ = np.arange(NB)
    pre = (i_ % 2) * 32 + i_ // 2
    pim = pre + 64
    G = np.zeros((32, 128, 128))
    for f in range(32):
        if f == 0:
            for i in range(NB):
                G[0, pre[i], pre] = Wr[:, i, 0]
                G[0, pim[i], pim] = Wr[:, i, 32]
        else:
            for i in range(NB):
                G[f, pre[i], pre] = Wr[:, i, f]
                G[f, pim[i], pre] = -Wi[:, i, f]
                G[f, pre[i], pim] = Wi[:, i, f]
                G[f, pim[i], pim] = Wr[:, i, f]
    return A2, G, B2


# ---------------------------------------------------------------- bass trace
def _trace_nc():
    import concourse.bass as bass  # noqa: F401
    import concourse.mybir as mybir
    import concourse.tile as tile
    from concourse import bacc

    f32 = mybir.dt.float32
    f16 = mybir.dt.float16

    nc = bacc.Bacc("TRN2", target_bir_lowering=False, debug=False,
                   num_devices=NCORES)
    x_h = nc.dram_tensor("x_shard", [128, NJ * T], f16, kind="ExternalInput").ap()
    a_h = nc.dram_tensor("a2_mat", [128, 128], f16, kind="ExternalInput").ap()
    g_h = nc.dram_tensor("g_mats", [128, 32 * 128], f16, kind="ExternalInput").ap()
    b_h = nc.dram_tensor("b2_mat", [128, 128], f16, kind="ExternalInput").ap()
    y_h = nc.dram_tensor("y_shard", [128, NJ * T], f16, kind="ExternalOutput").ap()

    cp_ix = [0]
    shuf_ix = [0]
    unshuf_ix = [0]

    with tile.TileContext(nc) as tc, ExitStack() as ctx:
        wpool = ctx.enter_context(tc.tile_pool(name="weights", bufs=1))
        dpool = ctx.enter_context(tc.tile_pool(name="data", bufs=1))
        y2p = ctx.enter_context(tc.tile_pool(name="y2p", bufs=3))
        big_ps = ctx.enter_context(tc.tile_pool(name="big_ps", bufs=2, space="PSUM"))

        def copyback(out_ap, in_ap):
            # PSUM readers: DVE + Act only
            if cp_ix[0] % 2 == 0:
                nc.vector.tensor_copy(out_ap, in_ap)
            else:
                nc.scalar.copy(out_ap, in_ap)
            cp_ix[0] += 1

        def shuf_dma(dst, src):
            eng = (nc.sync, nc.scalar, nc.gpsimd)[shuf_ix[0] % 3]
            shuf_ix[0] += 1
            return eng.dma_start(dst, src)

        def unshuf_dma(dst, src):
            eng = (nc.gpsimd, nc.sync, nc.scalar)[unshuf_ix[0] % 3]
            unshuf_ix[0] += 1
            return eng.dma_start(dst, src)

        a2t = wpool.tile([128, 128], f16)
        nc.gpsimd.dma_start(a2t[:], a_h[:])
        gt = wpool.tile([128, 32 * 128], f16)
        nc.scalar.dma_start(gt[:], g_h[:])
        b2t = wpool.tile([128, 128], f16)
        nc.gpsimd.dma_start(b2t[:], b_h[:])

        # ---- load x (8 streaming chunks, 4 block-pairs each, 3 queues)
        xt = dpool.tile([128, NJ * T], f16, tag="xt")
        for ch in range(8):
            eng = (nc.sync, nc.scalar, nc.gpsimd)[ch % 3]
            eng.dma_start(xt[:, ch * 4 * T:(ch + 1) * 4 * T],
                          x_h[:, ch * 4 * T:(ch + 1) * 4 * T])

        # ---- S1: 32 matmuls, shared stationary A2; 4-bank merged copybacks
        x1 = dpool.tile([128, NJ * T], f16, tag="x1")
        for grp in range(NJ // 4):
            ps = big_ps.tile([128, 4 * T], f32, tag="ps")
            for k in range(4):
                j = 4 * grp + k
                nc.tensor.matmul(ps[:, k * T:(k + 1) * T], a2t[:],
                                 xt[:, j * T:(j + 1) * T], start=True, stop=True)
            copyback(x1[:, (4 * grp) * T:(4 * grp + 4) * T], ps[:])

        # ---- shuffle: x1 partitions are phi-major (p = 2*phi + b), so each
        # freq-pair f reads 4 contiguous partitions [4f, 4f+4) fanned out to
        # 128 dst partitions (c1*64 + b*32 + j).
        x2 = dpool.tile([128, NJ * T], f16, tag="x2")
        for f in range(32):
            shuf_dma(x2[:, f * T:(f + 1) * T], x1[4 * f:4 * f + 4, :])

        # ---- S2 + unshuffle (y3 partitions phi-major: p = 2*phi + b2)
        y3 = dpool.tile([128, NJ * T], f16, tag="y3")
        for grp in range(8):
            ps = big_ps.tile([128, 4 * T], f32, tag="ps")
            for k in range(4):
                f = 4 * grp + k
                nc.tensor.matmul(ps[:, k * T:(k + 1) * T],
                                 gt[:, f * 128:(f + 1) * 128],
                                 x2[:, f * T:(f + 1) * T], start=True, stop=True)
            y2c = y2p.tile([128, 4 * T], f16, tag="y2")
            copyback(y2c[:], ps[:])
            for k in range(4):
                f = 4 * grp + k
                unshuf_dma(y3[4 * f:4 * f + 4, :], y2c[:, k * T:(k + 1) * T])

        # ---- S3: 32 matmuls, shared stationary B2; store per group
        yo = dpool.tile([128, NJ * T], f16, tag="yo")
        for grp in range(8):
            ps = big_ps.tile([128, 4 * T], f32, tag="ps")
            for k in range(4):
                j = 4 * grp + k
                nc.tensor.matmul(ps[:, k * T:(k + 1) * T], b2t[:],
                                 y3[:, j * T:(j + 1) * T], start=True, stop=True)
            copyback(yo[:, (4 * grp) * T:(4 * grp + 4) * T], ps[:])
            eng = (nc.sync, nc.scalar, nc.gpsimd)[grp % 3]
            eng.dma_start(y_h[:, (4 * grp) * T:(4 * grp + 4) * T],
                          yo[:, (4 * grp) * T:(4 * grp + 4) * T])

    nc.compile()
    return nc


_CACHE = {}


def make_in_maps(x, circulant_params, channel_weights):
    xf = np.ascontiguousarray(np.asarray(x, np.float32)).reshape(-1, FEAT)
    assert xf.shape[0] == NCORES * T, f"unexpected token count {xf.shape}"
    A2, G, B2 = _build_matrices(circulant_params, channel_weights)
    a2_f16 = A2.astype(np.float16)
    b2_f16 = B2.astype(np.float16)
    g_f16 = np.ascontiguousarray(
        G.transpose(1, 0, 2).reshape(128, 32 * 128).astype(np.float16))
    xf16 = xf.astype(np.float16)
    maps = []
    for c in range(NCORES):
        xs = xf16[c * T:(c + 1) * T].T                       # [4096, 512]
        xt = np.ascontiguousarray(
            xs.reshape(NJ, 2, BLK, T).transpose(1, 2, 0, 3).reshape(128, NJ * T))
        maps.append({
            "x_shard": xt,
            "a2_mat": a2_f16,
            "g_mats": g_f16,
            "b2_mat": b2_f16,
        })
    return maps


def kernel(x, circulant_params, channel_weights):
    from concourse.bass_utils import run_bass_kernel_spmd

    x = np.ascontiguousarray(np.asarray(x, np.float32))
    orig_shape = x.shape

    if "nc" not in _CACHE:
        _CACHE["nc"] = _trace_nc()
    nc = _CACHE["nc"]

    in_maps = make_in_maps(x, circulant_params, channel_weights)
    res = run_bass_kernel_spmd(nc, in_maps, core_ids=list(range(NCORES)))
    outs = []
    for c in range(NCORES):
        yo = res.results[c]["y_shard"]                       # [128, 32*T] f16
        yv = yo.reshape(2, BLK, NJ, T).transpose(3, 2, 0, 1).reshape(T, FEAT)
        outs.append(yv)
    y = np.concatenate(outs, axis=0)
    return y.astype(np.float32).reshape(orig_shape)


# revision 12
# speedup vs baseline: 1.2210x; 1.2009x over previous
"""Trainium2 Bass kernel for nn_FFTChainMatrix (block-circulant matmul via 64-pt rFFT).

W=8 design: 4 blocks per 128-partition tile, 8-partition frequency strips.

y = x @ W.T with W 4096x4096 block-circulant (64x64 grid of 64x64 circulant
blocks).  FFT-domain pipeline per 512-token core shard, all tiles [128, *]:

  load    x as [128=(b,chat), (kq,g)-cols] streaming HBM layout
  S1      rfft: 64 MMs (16 g x 2 mu outs, 2-way kq accum), 4 shared stationaries
  shuf    8-partition strips -> freq-major: 32 SBUF DMAs (8-engine fan-out)
  S2      per-freq-pair contraction over blocks: 32 MMs
  unshuf  freq-major -> strips: 32 SBUF DMAs
  S3      irfft: 64 MMs (16 g' x 2 h outs, 2-way mu accum), 4 shared stationaries
  store   yo [128, 32*T] -> HBM; host un-permutes/transposes.

Sharding: data-parallel over tokens, 4096 tokens -> 8 cores x 512.
"""

from contextlib import ExitStack

import numpy as np

BLK = 64
NB = 64           # blocks per side
T = 512           # tokens per core
NCORES = 8
FEAT = 4096
NT = 32           # column tiles of [128, T]


# ---------------------------------------------------------------- host math
def _build_afft():
    r = np.arange(BLK)
    A = np.zeros((BLK, BLK))
    A[0, :] = 1.0
    A[1, :] = (-1.0) ** r
    for p in range(1, 32):
        A[2 * p, :] = np.cos(2 * np.pi * p * r / BLK)
        A[2 * p + 1, :] = -np.sin(2 * np.pi * p * r / BLK)
    return A


def _build_bifft():
    r = np.arange(BLK)
    B = np.zeros((BLK, BLK))
    B[:, 0] = 1.0 / BLK
    B[:, 1] = ((-1.0) ** r) / BLK
    for p in range(1, 32):
        B[:, 2 * p] = 2.0 * np.cos(2 * np.pi * p * r / BLK) / BLK
        B[:, 2 * p + 1] = -2.0 * np.sin(2 * np.pi * p * r / BLK) / BLK
    return B


def _build_matrices(circulant_params, channel_weights):
    """A4 [2,2,128,128], G [32,128,128], B4 [2,2,128,128] float64."""
    Afft = _build_afft()
    Bifft = _build_bifft()
    A4 = np.zeros((2, 2, 128, 128))
    for kq in range(2):
        for mu in range(2):
            for b in range(4):
                for ch in range(32):
                    A4[kq, mu, b * 32 + ch, np.arange(32) * 4 + b] = \
                        Afft[mu * 32 + np.arange(32), kq * 32 + ch]
    B4 = np.zeros((2, 2, 128, 128))
    for mu in range(2):
        for h in range(2):
            for ph in range(32):
                for btil in range(2):
                    b2 = 2 * h + btil
                    B4[mu, h, ph * 4 + b2, btil * 64 + np.arange(64)] = \
                        Bifft[np.arange(64), mu * 32 + ph]

    c_w = np.einsum("m,moid->oid",
                    np.asarray(channel_weights, np.float64),
                    np.asarray(circulant_params, np.float64))
    Chat = np.fft.rfft(c_w, axis=-1)
    Wr, Wi = Chat.real, Chat.imag
    i_ = np.arange(NB)
    qre = (i_ % 4) * 16 + i_ // 4
    qim = qre + 64
    G = np.zeros((32, 128, 128))
    for p in range(32):
        if p == 0:
            for i in range(NB):
                G[0, qre[i], qre] = Wr[:, i, 0]
                G[0, qim[i], qim] = Wr[:, i, 32]
        else:
            for i in range(NB):
                G[p, qre[i], qre] = Wr[:, i, p]
                G[p, qim[i], qre] = -Wi[:, i, p]
                G[p, qre[i], qim] = Wi[:, i, p]
                G[p, qim[i], qim] = Wr[:, i, p]
    return A4, G, B4


# ---------------------------------------------------------------- bass trace
def _trace_nc():
    import concourse.bass as bass  # noqa: F401
    import concourse.mybir as mybir
    import concourse.tile as tile
    from concourse import bacc

    f32 = mybir.dt.float32
    f16 = mybir.dt.float16

    nc = bacc.Bacc("TRN2", target_bir_lowering=False, debug=False,
                   num_devices=NCORES)
    x_h = nc.dram_tensor("x_shard", [128, NT * T], f16, kind="ExternalInput").ap()
    a_h = nc.dram_tensor("a4_mats", [128, 512], f16, kind="ExternalInput").ap()
    g_h = nc.dram_tensor("g_mats", [128, 32 * 128], f16, kind="ExternalInput").ap()
    b_h = nc.dram_tensor("b4_mats", [128, 512], f16, kind="ExternalInput").ap()
    y_h = nc.dram_tensor("y_shard", [128, NT * T], f16, kind="ExternalOutput").ap()

    cp_ix = [0]
    shuf_ix = [0]
    unshuf_ix = [0]

    with tile.TileContext(nc) as tc, ExitStack() as ctx:
        wpool = ctx.enter_context(tc.tile_pool(name="weights", bufs=1))
        dpool = ctx.enter_context(tc.tile_pool(name="data", bufs=1))
        y2p = ctx.enter_context(tc.tile_pool(name="y2p", bufs=3))
        big_ps = ctx.enter_context(tc.tile_pool(name="big_ps", bufs=2, space="PSUM"))

        def copyback(out_ap, in_ap):
            # PSUM readers: DVE + Act only
            if cp_ix[0] % 2 == 0:
                nc.vector.tensor_copy(out_ap, in_ap)
            else:
                nc.scalar.copy(out_ap, in_ap)
            cp_ix[0] += 1

        def shuf_dma(dst, src):
            eng = (nc.sync, nc.scalar, nc.gpsimd)[shuf_ix[0] % 3]
            shuf_ix[0] += 1
            return eng.dma_start(dst, src)

        def unshuf_dma(dst, src):
            eng = (nc.gpsimd, nc.sync, nc.scalar)[unshuf_ix[0] % 3]
            unshuf_ix[0] += 1
            return eng.dma_start(dst, src)

        a4t = wpool.tile([128, 512], f16)
        nc.gpsimd.dma_start(a4t[:], a_h[:])
        gt = wpool.tile([128, 32 * 128], f16)
        nc.scalar.dma_start(gt[:], g_h[:])
        b4t = wpool.tile([128, 512], f16)
        nc.gpsimd.dma_start(b4t[:], b_h[:])

        # ---- load x (8 streaming chunks of [128, 2048], 3 queues)
        xt = dpool.tile([128, NT * T], f16, tag="xt")
        for ch in range(8):
            eng = (nc.sync, nc.scalar, nc.gpsimd)[ch % 3]
            eng.dma_start(xt[:, ch * 4 * T:(ch + 1) * 4 * T],
                          x_h[:, ch * 4 * T:(ch + 1) * 4 * T])

        # ---- S1: 64 MMs; output col tiles (mu*16+g); 4-bank copybacks
        # x1 partitions: p = phihat*4 + b (per mu column block)
        x1 = dpool.tile([128, NT * T], f16, tag="x1")
        for mu in range(2):
            for gq in range(4):
                ps = big_ps.tile([128, 4 * T], f32, tag="ps")
                for k in range(4):
                    g = 4 * gq + k
                    for kq in range(2):
                        nc.tensor.matmul(
                            ps[:, k * T:(k + 1) * T],
                            a4t[:, (kq * 2 + mu) * 128:(kq * 2 + mu + 1) * 128],
                            xt[:, (kq * 16 + g) * T:(kq * 16 + g + 1) * T],
                            start=(kq == 0), stop=(kq == 1))
                copyback(x1[:, (mu * 16 + 4 * gq) * T:(mu * 16 + 4 * gq + 4) * T],
                         ps[:])

        # ---- shuffle: strip [8fl, 8fl+8) x mu-col-block -> x2 tile f
        # x2[f] partitions: q = (c1*4+b)*16 + g
        x2 = dpool.tile([128, NT * T], f16, tag="x2")
        for f in range(32):
            mu, fl = f // 16, f % 16
            src = x1[8 * fl:8 * fl + 8, mu * 16 * T:(mu + 1) * 16 * T]
            shuf_dma(x2[:, f * T:(f + 1) * T], src)

        # ---- S2 + unshuffle (4 f per PSUM tile)
        y3 = dpool.tile([128, NT * T], f16, tag="y3")
        for grp in range(8):
            ps = big_ps.tile([128, 4 * T], f32, tag="ps")
            for k in range(4):
                f = 4 * grp + k
                nc.tensor.matmul(ps[:, k * T:(k + 1) * T],
                                 gt[:, f * 128:(f + 1) * 128],
                                 x2[:, f * T:(f + 1) * T], start=True, stop=True)
            y2c = y2p.tile([128, 4 * T], f16, tag="y2")
            copyback(y2c[:], ps[:])
            for k in range(4):
                f = 4 * grp + k
                mu, fl = f // 16, f % 16
                dst = y3[8 * fl:8 * fl + 8, mu * 16 * T:(mu + 1) * 16 * T]
                unshuf_dma(dst, y2c[:, k * T:(k + 1) * T])

        # ---- S3: 64 MMs; output col tiles tau' = g'*2 + h; store per group
        yo = dpool.tile([128, NT * T], f16, tag="yo")
        for q4 in range(8):
            ps = big_ps.tile([128, 4 * T], f32, tag="ps")
            for k in range(4):
                tau = 4 * q4 + k
                gp, h = tau // 2, tau % 2
                for mu in range(2):
                    nc.tensor.matmul(
                        ps[:, k * T:(k + 1) * T],
                        b4t[:, (mu * 2 + h) * 128:(mu * 2 + h + 1) * 128],
                        y3[:, (mu * 16 + gp) * T:(mu * 16 + gp + 1) * T],
                        start=(mu == 0), stop=(mu == 1))
            copyback(yo[:, (4 * q4) * T:(4 * q4 + 4) * T], ps[:])
            eng = (nc.sync, nc.scalar, nc.gpsimd)[q4 % 3]
            eng.dma_start(y_h[:, (4 * q4) * T:(4 * q4 + 4) * T],
                          yo[:, (4 * q4) * T:(4 * q4 + 4) * T])

    nc.compile()
    return nc


_CACHE = {}


def make_in_maps(x, circulant_params, channel_weights):
    xf = np.ascontiguousarray(np.asarray(x, np.float32)).reshape(-1, FEAT)
    assert xf.shape[0] == NCORES * T, f"unexpected token count {xf.shape}"
    A4, G, B4 = _build_matrices(circulant_params, channel_weights)
    # a4 cols: (kq*2+mu)*128
    a4_f16 = np.concatenate(
        [A4[kq, mu] for kq in range(2) for mu in range(2)], axis=1).astype(np.float16)
    b4_f16 = np.concatenate(
        [B4[mu, h] for mu in range(2) for h in range(2)], axis=1).astype(np.float16)
    g_f16 = np.ascontiguousarray(
        G.transpose(1, 0, 2).reshape(128, 32 * 128).astype(np.float16))
    xf16 = xf.astype(np.float16)
    maps = []
    for c in range(NCORES):
        xs = xf16[c * T:(c + 1) * T].T                       # [4096, 512]
        # xt[b*32+ch, (kq*16+g)*T+t] = x[(4g+b)*64+kq*32+ch, t]
        xt = np.ascontiguousarray(
            xs.reshape(16, 4, 2, 32, T).transpose(1, 3, 2, 0, 4).reshape(128, NT * T))
        maps.append({
            "x_shard": xt,
            "a4_mats": a4_f16,
            "g_mats": g_f16,
            "b4_mats": b4_f16,
        })
    return maps


def kernel(x, circulant_params, channel_weights):
    from concourse.bass_utils import run_bass_kernel_spmd

    x = np.ascontiguousarray(np.asarray(x, np.float32))
    orig_shape = x.shape

    if "nc" not in _CACHE:
        _CACHE["nc"] = _trace_nc()
    nc = _CACHE["nc"]

    in_maps = make_in_maps(x, circulant_params, channel_weights)
    res = run_bass_kernel_spmd(nc, in_maps, core_ids=list(range(NCORES)))
    outs = []
    for c in range(NCORES):
        yo = res.results[c]["y_shard"]                       # [128, 32*T] f16
        # y[t, (4g'+2h+btil)*64+d] = yo[btil*64+d, (2g'+h)*T+t]
        yv = yo.reshape(2, BLK, 16, 2, T).transpose(4, 2, 3, 0, 1).reshape(T, FEAT)
        outs.append(yv)
    y = np.concatenate(outs, axis=0)
    return y.astype(np.float32).reshape(orig_shape)
